# revision 9
# baseline (speedup 1.0000x reference)
"""EnergyAttention kernel for Trainium2 (Bass/Tile), 8-core data parallel.

Reference semantics:
    energy = einsum('bcd,d->bc', inputs, W[0]) + b[0]
    attn   = softmax(energy, axis=1)
    idx    = top_k(attn, 16).indices          # [B, K] descending
    out    = take_along_axis(inputs, idx)     # [B, K, D]

Key simplification: softmax is strictly monotonic per row and the bias is a
per-row constant shift, so top_k(attn) == top_k(energy) == top_k(dot(x, W)).
The output is just gathered input rows; no softmax/bias needed on device.

Per-core plan (B/8 = 32 batch rows, N = 32*512 = 16384 clip rows):
  1. Stream all 16384 rows through SBUF in 128-row tiles; one fused DVE
     tensor_tensor_reduce per tile computes 128 dot products with W.
  2. Rows are loaded in a permuted order (n = 512i + 128A + 32Q + j at
     partition 32A+j of column f=32Q+i) so that after one DVE 32x32
     stream-transpose and a 4-way partition fold the energies land as
     E2[b, c] with b on partitions and all 512 c in the free dim.
  3. Top-16 per row = two rounds of max/max_index (top-8 each) with a
     match_replace(-inf) between rounds.
  4. indirect_dma_start gathers the selected rows from DRAM; plain DMA
     stores them to the output.
"""

import numpy as np

import concourse.bacc as bacc
import concourse.bass as bass
import concourse.mybir as mybir
import concourse.tile as tile
from concourse.bass_utils import run_bass_kernel_spmd

B, C, D, K = 256, 512, 1024, 16
N_CORES = 8
BPC = B // N_CORES          # batch rows per core
N = BPC * C                 # clip rows per core
G = 1                       # energy columns per load DMA

F32 = mybir.dt.float32
U32 = mybir.dt.uint32

NEG_INF = -3.0e38


def build_module() -> bass.Bass:
    nc = bacc.Bacc(None)
    x = nc.declare_dram_parameter("x", [N, D], F32, isOutput=False)
    w = nc.declare_dram_parameter("w", [128, D], F32, isOutput=False)
    y = nc.declare_dram_parameter("y", [BPC, K, D], F32, isOutput=True)

    with tile.TileContext(nc) as tc:
        with (
            tc.tile_pool(name="xin", bufs=8) as xin_pool,
            tc.tile_pool(name="scr", bufs=2) as scr_pool,
            tc.tile_pool(name="small", bufs=1) as small_pool,
            tc.tile_pool(name="gath", bufs=4) as gath_pool,
        ):
            w_sb = small_pool.tile([128, D], F32, tag="w")
            nc.sync.dma_start(out=w_sb[:], in_=w[:])

            # ST[p, f] = energy of row n = 512*(f%32) + 128*(p//32) + 32*(f//32) + p%32
            st = small_pool.tile([128, 128], F32, tag="st")

            # x rows viewed as n = ((i*4 + A)*4 + Q)*32 + j, reordered (A j i Q d)
            x5 = x[:].rearrange("(i A Q j) d -> A j i Q d", i=32, A=4, Q=4, j=32)

            for q in range(4):
                for i in range(32):
                    f = 32 * q + i
                    xt = xin_pool.tile([128, D], F32, tag="xt")
                    nc.sync.dma_start(out=xt[:], in_=x5[:, :, i, q, :])
                    # fused dot(x, W): out = (x*1 + 0)*W, accum = sum over free
                    sc = scr_pool.tile([128, D], F32, tag="sc")
                    nc.vector.affine_mul_reduce(
                        out=sc[:],
                        accum_out=st[:, f : f + 1],
                        in0=xt[:],
                        in1=w_sb[:],
                        scale=1.0,
                        bias=0.0,
                    )

            # 32x32 block transpose, then fold partitions 32A+i -> i
            st2 = small_pool.tile([128, 128], F32, tag="st2")
            nc.vector.transpose(out=st2[:], in_=st[:])
            e2 = small_pool.tile([BPC, C], F32, tag="e2")
            for a in range(4):
                nc.sync.dma_start(
                    out=e2[:, 128 * a : 128 * (a + 1)],
                    in_=st2[32 * a : 32 * (a + 1), :],
                )

            # top-16 indices per batch row: two top-8 rounds
            m1 = small_pool.tile([BPC, 8], F32, tag="m1")
            m2 = small_pool.tile([BPC, 8], F32, tag="m2")
            idx = small_pool.tile([BPC, K], U32, tag="idx")
            e2b = small_pool.tile([BPC, C], F32, tag="e2b")
            nc.vector.max(out=m1[:], in_=e2[:])
            nc.vector.max_index(out=idx[:, 0:8], in_max=m1[:], in_values=e2[:])
            nc.vector.match_replace(
                out=e2b[:], in_to_replace=m1[:], in_values=e2[:], imm_value=NEG_INF
            )
            nc.vector.max(out=m2[:], in_=e2b[:])
            nc.vector.max_index(out=idx[:, 8:16], in_max=m2[:], in_values=e2b[:])

            # global row offsets: noff[b, k] = 512*b + idx[b, k].
            # idx < 512 and the base has the low 9 bits clear, so OR == ADD.
            nbase_sb = small_pool.tile([BPC, K], U32, tag="nbase")
            nc.gpsimd.iota(
                out=nbase_sb[:], pattern=[[0, K]], base=0, channel_multiplier=C
            )
            noff = small_pool.tile([BPC, K], U32, tag="noff")
            nc.vector.tensor_tensor(
                out=noff[:],
                in0=idx[:],
                in1=nbase_sb[:],
                op=mybir.AluOpType.bitwise_or,
            )

            # gather selected rows from DRAM, store to output
            for k in range(K):
                gt = gath_pool.tile([BPC, D], F32, tag="gt")
                nc.gpsimd.indirect_dma_start(
                    out=gt[:],
                    out_offset=None,
                    in_=x[:],
                    in_offset=bass.IndirectOffsetOnAxis(ap=noff[:, k : k + 1], axis=0),
                )
                nc.sync.dma_start(out=y[:, k, :], in_=gt[:])

    nc.finalize()
    return nc


_NC_CACHE: list = []


def _get_nc() -> bass.Bass:
    if not _NC_CACHE:
        _NC_CACHE.append(build_module())
    return _NC_CACHE[0]


def make_in_maps(inputs: np.ndarray, W: np.ndarray) -> list[dict]:
    w_rep = np.ascontiguousarray(
        np.broadcast_to(W.reshape(1, D).astype(np.float32, copy=False), (128, D))
    )
    return [
        {
            "x": np.ascontiguousarray(
                inputs[c * BPC : (c + 1) * BPC].reshape(N, D).astype(np.float32, copy=False)
            ),
            "w": w_rep,
        }
        for c in range(N_CORES)
    ]


def kernel(**inputs) -> np.ndarray:
    x_full = np.asarray(inputs["inputs"], dtype=np.float32)
    W = np.asarray(inputs["W"], dtype=np.float32)
    assert x_full.shape == (B, C, D), x_full.shape
    assert int(np.asarray(inputs["topk"])) == K
    nc = _get_nc()
    res = run_bass_kernel_spmd(nc, make_in_maps(x_full, W), core_ids=list(range(N_CORES)))
    return np.concatenate([res.results[c]["y"] for c in range(N_CORES)], axis=0).reshape(
        B, K, D
    )


# revision 11
# speedup vs baseline: 3.2252x; 3.2252x over previous
"""EnergyAttention kernel for Trainium2 (Bass/Tile), 8-core data parallel.

Reference semantics:
    energy = einsum('bcd,d->bc', inputs, W[0]) + b[0]
    attn   = softmax(energy, axis=1)
    idx    = top_k(attn, 16).indices          # [B, K] descending
    out    = take_along_axis(inputs, idx)     # [B, K, D]

Key simplification: softmax is strictly monotonic per row and the bias is a
per-row constant shift, so top_k(attn) == top_k(energy) == top_k(dot(x, W)).
The output is just gathered input rows; no softmax/bias needed on device.

Per-core plan (B/8 = 32 batch rows, N = 32*512 = 16384 clip rows):
  1. Stream the rows through SBUF in natural-order 128-row tiles (contiguous
     DRAM -> full DMA bandwidth); one fused DVE affine_mul_reduce per tile
     computes 128 dot products with W into column t of E1[128, 128].
  2. E1[p, t] = energy[b = t//4, c = 128*(t%4) + p]. One TensorE transpose
     (via identity) -> PSUM, copy back to SBUF, then a 4-way partition fold
     (stride-4 partition DMA) lands E2[b, c] with b on partitions and all
     512 c in the free dim.
  3. Top-16 per row = two rounds of max/max_index (top-8 each) with a
     match_replace(-inf) between rounds.
  4. indirect_dma_start gathers the selected rows from DRAM; plain DMA
     stores them to the output.
"""

import numpy as np

import concourse.bacc as bacc
import concourse.bass as bass
import concourse.mybir as mybir
import concourse.tile as tile
from concourse.bass_utils import run_bass_kernel_spmd
from concourse.masks import make_identity

B, C, D, K = 256, 512, 1024, 16
N_CORES = 8
BPC = B // N_CORES          # batch rows per core
N = BPC * C                 # clip rows per core
NT = N // 128               # 128 row-tiles per core

F32 = mybir.dt.float32
U32 = mybir.dt.uint32

NEG_INF = -3.0e38


def build_module() -> bass.Bass:
    nc = bacc.Bacc(None)
    x = nc.declare_dram_parameter("x", [N, D], F32, isOutput=False)
    w = nc.declare_dram_parameter("w", [128, D], F32, isOutput=False)
    y = nc.declare_dram_parameter("y", [BPC, K, D], F32, isOutput=True)

    with tile.TileContext(nc) as tc:
        with (
            tc.tile_pool(name="xin", bufs=12) as xin_pool,
            tc.tile_pool(name="scr", bufs=2) as scr_pool,
            tc.tile_pool(name="small", bufs=1) as small_pool,
            tc.tile_pool(name="gath", bufs=4) as gath_pool,
            tc.tile_pool(name="ps", bufs=1, space="PSUM") as ps_pool,
        ):
            w_sb = small_pool.tile([128, D], F32, tag="w")
            nc.sync.dma_start(out=w_sb[:], in_=w[:])
            ident = small_pool.tile([128, 128], F32, tag="ident")
            make_identity(nc, ident[:])

            # E1[p, t] = energy of row 128*t + p
            e1 = small_pool.tile([128, NT], F32, tag="e1")

            for t in range(NT):
                xt = xin_pool.tile([128, D], F32, tag="xt")
                nc.sync.dma_start(out=xt[:], in_=x[128 * t : 128 * (t + 1), :])
                # fused dot(x, W): out = (x*1 + 0)*W, accum = sum over free
                sc = scr_pool.tile([128, D], F32, tag="sc")
                nc.vector.affine_mul_reduce(
                    out=sc[:],
                    accum_out=e1[:, t : t + 1],
                    in0=xt[:],
                    in1=w_sb[:],
                    scale=1.0,
                    bias=0.0,
                )

            # e1[p, 4b + chi] = energy[b, 128*chi + p]. Four TensorE transposes
            # of the stride-4 column slices land E2[b, c] directly in PSUM.
            ps2 = ps_pool.tile([BPC, C], F32, tag="ps2")
            e1r = e1[:].rearrange("p (b chi) -> p chi b", chi=4)
            for chi in range(4):
                nc.tensor.transpose(
                    out=ps2[:, 128 * chi : 128 * (chi + 1)],
                    in_=e1r[:, chi, :],
                    identity=ident[:],
                )
            e2 = small_pool.tile([BPC, C], F32, tag="e2")
            nc.vector.tensor_copy(out=e2[:], in_=ps2[:])

            # top-16 indices per batch row: two top-8 rounds
            m1 = small_pool.tile([BPC, 8], F32, tag="m1")
            m2 = small_pool.tile([BPC, 8], F32, tag="m2")
            idx = small_pool.tile([BPC, K], U32, tag="idx")
            e2b = small_pool.tile([BPC, C], F32, tag="e2b")
            nc.vector.max(out=m1[:], in_=e2[:])
            nc.vector.max_index(out=idx[:, 0:8], in_max=m1[:], in_values=e2[:])
            nc.vector.match_replace(
                out=e2b[:], in_to_replace=m1[:], in_values=e2[:], imm_value=NEG_INF
            )
            nc.vector.max(out=m2[:], in_=e2b[:])
            nc.vector.max_index(out=idx[:, 8:16], in_max=m2[:], in_values=e2b[:])

            # global row offsets: noff[b, k] = 512*b + idx[b, k].
            # idx < 512 and the base has the low 9 bits clear, so OR == ADD.
            nbase_sb = small_pool.tile([BPC, K], U32, tag="nbase")
            nc.gpsimd.iota(
                out=nbase_sb[:], pattern=[[0, K]], base=0, channel_multiplier=C
            )
            noff = small_pool.tile([BPC, K], U32, tag="noff")
            nc.vector.tensor_tensor(
                out=noff[:],
                in0=idx[:],
                in1=nbase_sb[:],
                op=mybir.AluOpType.bitwise_or,
            )

            # gather selected rows from DRAM, store to output
            for k in range(K):
                gt = gath_pool.tile([BPC, D], F32, tag="gt")
                nc.gpsimd.indirect_dma_start(
                    out=gt[:],
                    out_offset=None,
                    in_=x[:],
                    in_offset=bass.IndirectOffsetOnAxis(ap=noff[:, k : k + 1], axis=0),
                )
                nc.sync.dma_start(out=y[:, k, :], in_=gt[:])

    nc.finalize()
    return nc


_NC_CACHE: list = []


def _get_nc() -> bass.Bass:
    if not _NC_CACHE:
        _NC_CACHE.append(build_module())
    return _NC_CACHE[0]


def make_in_maps(inputs: np.ndarray, W: np.ndarray) -> list[dict]:
    w_rep = np.ascontiguousarray(
        np.broadcast_to(W.reshape(1, D).astype(np.float32, copy=False), (128, D))
    )
    return [
        {
            "x": np.ascontiguousarray(
                inputs[c * BPC : (c + 1) * BPC].reshape(N, D).astype(np.float32, copy=False)
            ),
            "w": w_rep,
        }
        for c in range(N_CORES)
    ]


def kernel(**inputs) -> np.ndarray:
    x_full = np.asarray(inputs["inputs"], dtype=np.float32)
    W = np.asarray(inputs["W"], dtype=np.float32)
    assert x_full.shape == (B, C, D), x_full.shape
    assert int(np.asarray(inputs["topk"])) == K
    nc = _get_nc()
    res = run_bass_kernel_spmd(nc, make_in_maps(x_full, W), core_ids=list(range(N_CORES)))
    return np.concatenate([res.results[c]["y"] for c in range(N_CORES)], axis=0).reshape(
        B, K, D
    )


# revision 17
# speedup vs baseline: 3.3525x; 1.0395x over previous
"""EnergyAttention kernel for Trainium2 (Bass/Tile), 8-core data parallel.

Reference semantics:
    energy = einsum('bcd,d->bc', inputs, W[0]) + b[0]
    attn   = softmax(energy, axis=1)
    idx    = top_k(attn, 16).indices          # [B, K] descending
    out    = take_along_axis(inputs, idx)     # [B, K, D]

Key simplification: softmax is strictly monotonic per row and the bias is a
per-row constant shift, so top_k(attn) == top_k(energy) == top_k(dot(x, W)).
The output is just gathered input rows; no softmax/bias needed on device.

Per-core plan (B/8 = 32 batch rows, N = 32*512 = 16384 clip rows):
  1. Stream the rows through SBUF in natural-order 128-row tiles (contiguous
     DRAM -> full DMA bandwidth); one fused DVE affine_mul_reduce per tile
     computes 128 dot products with W into column t of E1[128, 128].
  2. E1[p, t] = energy[b = t//4, c = 128*(t%4) + p]. One TensorE transpose
     (via identity) -> PSUM, copy back to SBUF, then a 4-way partition fold
     (stride-4 partition DMA) lands E2[b, c] with b on partitions and all
     512 c in the free dim.
  3. Top-16 per row = two rounds of max/max_index (top-8 each) with a
     match_replace(-inf) between rounds.
  4. indirect_dma_start gathers the selected rows from DRAM; plain DMA
     stores them to the output.
"""

import numpy as np

import concourse.bacc as bacc
import concourse.bass as bass
import concourse.mybir as mybir
import concourse.tile as tile
from concourse.bass_utils import run_bass_kernel_spmd
from concourse.masks import make_identity

B, C, D, K = 256, 512, 1024, 16
N_CORES = 8
BPC = B // N_CORES          # batch rows per core
N = BPC * C                 # clip rows per core
NT = N // 128               # 128 row-tiles per core

F32 = mybir.dt.float32
U32 = mybir.dt.uint32

NEG_INF = -3.0e38


def build_module() -> bass.Bass:
    nc = bacc.Bacc(None)
    x = nc.declare_dram_parameter("x", [N, D], F32, isOutput=False)
    w = nc.declare_dram_parameter("w", [128, D], F32, isOutput=False)
    y = nc.declare_dram_parameter("y", [BPC, K, D], F32, isOutput=True)

    with tile.TileContext(nc) as tc:
        with (
            tc.tile_pool(name="xin", bufs=12) as xin_pool,
            tc.tile_pool(name="scr", bufs=2) as scr_pool,
            tc.tile_pool(name="small", bufs=1) as small_pool,
            tc.tile_pool(name="gath", bufs=4) as gath_pool,
            tc.tile_pool(name="ps", bufs=1, space="PSUM") as ps_pool,
        ):
            w_sb = small_pool.tile([128, D], F32, tag="w")
            nc.sync.dma_start(out=w_sb[:], in_=w[:])
            ident = small_pool.tile([128, 128], F32, tag="ident")
            make_identity(nc, ident[:])
            # tiled identity: it[p, m] = 1 iff m % 16 == p  (p < 16)
            # replicates a [16, S] block to all 128 partitions via matmul
            it16 = small_pool.tile([16, 128], F32, tag="it16")
            nc.gpsimd.memset(it16[:], 0.0)
            for c8 in range(8):
                nc.gpsimd.affine_select(
                    out=it16[:, 16 * c8 : 16 * (c8 + 1)],
                    in_=it16[:, 16 * c8 : 16 * (c8 + 1)],
                    compare_op=mybir.AluOpType.not_equal,
                    fill=1.0,
                    base=0,
                    pattern=[[-1, 16]],
                    channel_multiplier=1,
                )

            # E1[p, t] = energy of row 128*t + p
            e1 = small_pool.tile([128, NT], F32, tag="e1")

            for t in range(NT):
                xt = xin_pool.tile([128, D], F32, tag="xt")
                nc.sync.dma_start(out=xt[:], in_=x[128 * t : 128 * (t + 1), :])
                # fused dot(x, W): out = (x*1 + 0)*W, accum = sum over free
                sc = scr_pool.tile([128, D], F32, tag="sc")
                nc.vector.affine_mul_reduce(
                    out=sc[:],
                    accum_out=e1[:, t : t + 1],
                    in0=xt[:],
                    in1=w_sb[:],
                    scale=1.0,
                    bias=0.0,
                )

            # e1[p, 4b + chi] = energy[b, 128*chi + p]. Four TensorE transposes
            # of the stride-4 column slices land E2[b, c] directly in PSUM.
            ps2 = ps_pool.tile([BPC, C], F32, tag="ps2")
            e1r = e1[:].rearrange("p (b chi) -> p chi b", chi=4)
            for chi in range(4):
                nc.tensor.transpose(
                    out=ps2[:, 128 * chi : 128 * (chi + 1)],
                    in_=e1r[:, chi, :],
                    identity=ident[:],
                )
            e2 = small_pool.tile([BPC, C], F32, tag="e2")
            nc.vector.tensor_copy(out=e2[:], in_=ps2[:])

            # top-16 indices per batch row: two top-8 rounds
            m1 = small_pool.tile([BPC, 8], F32, tag="m1")
            m2 = small_pool.tile([BPC, 8], F32, tag="m2")
            idx = small_pool.tile([BPC, K], U32, tag="idx")
            e2b = small_pool.tile([BPC, C], F32, tag="e2b")
            nc.vector.max(out=m1[:], in_=e2[:])
            nc.vector.max_index(out=idx[:, 0:8], in_max=m1[:], in_values=e2[:])
            nc.vector.match_replace(
                out=e2b[:], in_to_replace=m1[:], in_values=e2[:], imm_value=NEG_INF
            )
            nc.vector.max(out=m2[:], in_=e2b[:])
            nc.vector.max_index(out=idx[:, 8:16], in_max=m2[:], in_values=e2b[:])

            # global row offsets in f32 (exact below 2^24):
            # noff[b, k] = 512*b + idx[b, k]
            nbase_u = small_pool.tile([BPC, 1], U32, tag="nbase_u")
            nc.gpsimd.iota(
                out=nbase_u[:], pattern=[[0, 1]], base=0, channel_multiplier=C
            )
            nbase_f = small_pool.tile([BPC, 1], F32, tag="nbase_f")
            nc.vector.tensor_copy(out=nbase_f[:], in_=nbase_u[:])
            idx_f = small_pool.tile([BPC, K], F32, tag="idx_f")
            nc.vector.tensor_copy(out=idx_f[:], in_=idx[:])
            noff_f = small_pool.tile([BPC, K], F32, tag="noff_f")
            nc.vector.tensor_scalar(
                out=noff_f[:],
                in0=idx_f[:],
                scalar1=nbase_f[:, 0:1],
                scalar2=None,
                op0=mybir.AluOpType.add,
            )

            # dma_gather wants idxs[k, b] (int16) tiled 8x down 128 partitions
            # (one copy per Q7 core): PE-transpose then PE-replicate.
            ps_t = ps_pool.tile([K, BPC], F32, tag="ps_t")
            nc.tensor.transpose(
                out=ps_t[:], in_=noff_f[:], identity=ident[0:BPC, 0:BPC]
            )
            nofft = small_pool.tile([K, BPC], F32, tag="nofft")
            nc.vector.tensor_copy(out=nofft[:], in_=ps_t[:])
            ps_r = ps_pool.tile([128, BPC], F32, tag="ps_r")
            nc.tensor.matmul(out=ps_r[:], lhsT=it16[:], rhs=nofft[:])
            idxs16 = small_pool.tile([128, BPC], mybir.dt.int16, tag="idxs16")
            nc.vector.tensor_copy(out=idxs16[:], in_=ps_r[:])

            # one fused gather of all 512 selected rows; item i = 16*b + k
            # lands at go[i % 128, i // 128, :]
            NI = BPC * K
            go = gath_pool.tile([128, NI // 128, D], F32, tag="go")
            nc.gpsimd.dma_gather(
                out_ap=go[:],
                in_ap=x[:],
                idxs_ap=idxs16[:],
                num_idxs=NI,
                num_idxs_reg=NI,
                elem_size=D,
            )
            y_r = y[:].rearrange("b k d -> (b k) d").rearrange(
                "(j p) d -> p j d", p=128
            )
            nc.sync.dma_start(out=y_r, in_=go[:])

    nc.finalize()
    return nc


_NC_CACHE: list = []


def _get_nc() -> bass.Bass:
    if not _NC_CACHE:
        _NC_CACHE.append(build_module())
    return _NC_CACHE[0]


def make_in_maps(inputs: np.ndarray, W: np.ndarray) -> list[dict]:
    w_rep = np.ascontiguousarray(
        np.broadcast_to(W.reshape(1, D).astype(np.float32, copy=False), (128, D))
    )
    return [
        {
            "x": np.ascontiguousarray(
                inputs[c * BPC : (c + 1) * BPC].reshape(N, D).astype(np.float32, copy=False)
            ),
            "w": w_rep,
        }
        for c in range(N_CORES)
    ]


def kernel(**inputs) -> np.ndarray:
    x_full = np.asarray(inputs["inputs"], dtype=np.float32)
    W = np.asarray(inputs["W"], dtype=np.float32)
    assert x_full.shape == (B, C, D), x_full.shape
    assert int(np.asarray(inputs["topk"])) == K
    nc = _get_nc()
    res = run_bass_kernel_spmd(nc, make_in_maps(x_full, W), core_ids=list(range(N_CORES)))
    return np.concatenate([res.results[c]["y"] for c in range(N_CORES)], axis=0).reshape(
        B, K, D
    )
